# revision 5
# baseline (speedup 1.0000x reference)
"""Trainium2 Bass kernel for NormalAttention (embedded gaussian) — fp8 rev.

Per-core (B=8 data parallel, one sample per core), C=256, Ck=64, N=2304:
    q = Wq x; k = Wk x; e[i,j] = q_i . k_j; F = exp(e)
    out = (Wg Wv x / rowsum(F)) @ F + bg     (gamma conv folded into V)

vs the bf16 baseline (82us; this kernel: ~64.5us cost-model):
  - F stored as fp8e4m3 straight out of the ACT exp; the big att matmul
    runs in fp8 DoubleRow mode (256-deep contraction per instruction, 0.5
    cyc/col = 4x bf16). V is split v = v_hi + v_lo into two fp8 tensors
    (exact to ~0.1%), so att = (v_hi + v_lo) @ F leaves only F's
    quantization error (~1.3e-2 rel, within the 2e-2 budget). v is scaled
    by 4096/s_i (folds the softmax row norm and keeps fp8 in range); the
    stores descale by 1/4096.
  - row sums: B-half (1280) via expB's ACT accum_out (fast path for the
    v-split); A-half via a GPSIMD pairwise add + one DVE reduce, keeping
    ACT at its 2477ns/chunk floor. Last two chunks accum both halves so
    the final v-split (pair-8 critical path) is short.
  - PSUM: [energy A: 2 banks][energy B: 3][3 att lanes]. The 9 att
    output groups rotate through the lanes in tuned windows; partials
    flush to SBUF bf16 and reload later via an identity matmul (PE).
    V projections and deferred Q-tile projections borrow lane banks in
    window gaps. Tail: 4 groups reload into the freed energy regions
    (bank-separate zero regions) and pre-run while the last exp is still
    going; consolidated multi-group DMAs minimize serial HWDGE time.
  - output in bf16 (halves the store DMA traffic; +0.2% error).
"""

import os
import sys

sys.path.insert(0, "/opt/trn_rl_repo")
os.environ["BASS_NEVER_TRACE"] = "1"

_jp = os.environ.get("JAX_PLATFORMS")
if _jp and "axon" not in _jp and "jax" not in sys.modules:
    os.environ.pop("JAX_PLATFORMS", None)

import numpy as np
import ml_dtypes

import concourse.bass as bass
import concourse.mybir as mybir
import concourse.tile as tile
from concourse import bacc
from concourse.bass_utils import run_bass_kernel_spmd

B, C, CK, H, W = 8, 256, 64, 48, 48
N = H * W            # 2304
P = 128
NI = N // P          # 18 chunks
NPAIR = NI // 2      # 9 DoubleRow chunk pairs
NCORES = 8

BF16 = mybir.dt.bfloat16
F8 = mybir.dt.float8e4
F32 = mybir.dt.float32
AF = mybir.ActivationFunctionType
ALU = mybir.AluOpType
AX = mybir.AxisListType
DR = mybir.MatmulPerfMode.DoubleRow

SCALE = 4096.0
EA, EB = 1024, 1280
EA_SUBS = [(0, 512), (512, 512)]
EB_SUBS = [(0, 512), (512, 512), (1024, 256)]
QK_TILES = [(0, 512), (512, 512), (1024, 512), (1536, 512), (2048, 256)]
N_WARM = 5

# ---- att schedule ---------------------------------------------------------
G_STD = [(0, 0), (0, 512), (0, 1024), (0, 1536),
         (1, 0), (1, 512), (1, 1024), (1, 1536)]
G_COMP = 8
NG = 9

# group j-ranges: g0..g3 = oc0 j 0/512/1024/1536; g4..g7 = oc1 same; g8 =
# composite (both oc, j 2048:2304).
# group -> list of (lane, acquire_slot, release_slot); last window of a
# FINAL group completes in-phase (direct store), others flush partials.
WINDOWS = {
    0: [(0, 2, 7), (0, 12, 12)],
    1: [(1, 3, 8), (1, 13, 13)],
    5: [(2, 5, 11), (2, 14, 14)],
    3: [(0, 8, 11)],
    4: [(1, 9, 12)],
    7: [(2, 10, 13)],
    2: [(0, 12, 17)],
    6: [(1, 14, 17)],
    G_COMP: [(2, 15, 17)],
}
FINAL_GROUPS = (2, 6, G_COMP)       # complete in-phase, direct store
BURST_CAP = 4
# tail placement: the freed energy PSUM regions are 2 banks each, so four
# flushed groups run there concurrently (bank-separate zero regions) and
# pre-run their non-final pairs while the last exp still executes; the two
# remaining groups take lanes freed by the earliest final stores.
TAIL_BANK = {3: ("EA2", 0), 7: ("EA2", 512), 0: ("EB2", 0), 4: ("EB2", 512),
             1: ("L1", 0), 5: ("L2", 0)}
TAIL_EARLY = [3, 7, 0, 4]
# store engine split in the tail (DVE / ACT alternate)
STORE_ON_ACT = {2, 3, 4, 5}

# V-projection batch (2 chunks) -> (lane, slot); batch 0 is emitted in the
# head (its vsb chunks are consumed by chunk 0's v-split). A batch at slot
# s must satisfy s <= 2*bi - 1 so its writes precede the reader's emission.
VB_SLOT = {0: (0, -1), 6: (0, 1), 1: (1, 0), 7: (1, 2), 2: (1, 1),
           3: (2, 3), 4: (2, 0), 5: (2, 2), 8: (2, 3)}


def _plan_att():
    """Static att schedule: per-slot op lists + tail list.
    ops: ("open", g, tag, reload) | ("step", g, t, start, stop)
       | ("flush", g) | ("store", g)"""
    slot_ops = {s: [] for s in range(NI)}
    tail = []
    done_of = {}
    for g in range(NG):
        final = g in FINAL_GROUPS
        done = 0
        had_flush = False
        for wi, (lane, acq, rel) in enumerate(WINDOWS[g]):
            opened = False
            last_step_ref = None
            for s in range(acq, rel + 1):
                ready = NPAIR if s >= 17 else max(
                    0, min((s - 2) // 2 + 1, NPAIR))
                cap = 99 if s == rel else BURST_CAP
                n = min(ready - done, cap)
                if n <= 0 and not opened:
                    continue
                if not opened:
                    slot_ops[s].append(("open", g, f"L{lane}", 0, had_flush))
                    opened = True
                for i in range(n):
                    t = done + i
                    op = ["step", g, t, t == 0 and not had_flush, False]
                    slot_ops[s].append(op)
                    last_step_ref = op
                done += n
            assert opened and last_step_ref is not None, (g, wi)
            last_step_ref[4] = True  # stop accumulation at window end
            if not (final and wi == len(WINDOWS[g]) - 1):
                slot_ops[rel].append(("flush", g))
                had_flush = True
        done_of[g] = done
        if final:
            assert done == NPAIR, (g, done)
    slot_ops[17].sort(key=lambda op: 0 if op[1] == G_COMP else 1)
    # tail, phase (a): the four E-bank groups reload and run their
    # pre-pair-8 steps first — those only need old data, so they execute
    # while the last exp/v-split still runs.
    early = TAIL_EARLY
    late = [g for g in TAIL_BANK if g not in TAIL_EARLY]
    for g in early:
        tail.append(("open", g) + TAIL_BANK[g] + (True,))
        for t in range(done_of[g], NPAIR - 1):
            tail.append(("step", g, t, False, False))
    # (b): pair-8 steps + stores; finals store first so the late groups'
    # lanes free earliest. Consolidated DMAs fire once every group covering
    # a contiguous out range has stored: ("dma", oc_or_None, j0, j1) — oc
    # None spans both oc chunks.
    for g in (G_COMP, 2, 6):
        tail.append(("store", g))
    tail.append(("dma", None, 2048, 2304))         # g8
    for g in early:
        tail.append(("step", g, NPAIR - 1, False, True))
        tail.append(("store", g))
        if g == 3:
            tail.append(("dma", 0, 1024, 2048))    # g2 + g3
        if g == 7:
            tail.append(("dma", 1, 1024, 2048))    # g6 + g7
        if g == 4:
            tail.append(("dma", None, 0, 512))     # g0 + g4
    for g in late:
        tail.append(("open", g) + TAIL_BANK[g] + (True,))
        for t in range(done_of[g], NPAIR):
            tail.append(("step", g, t, False, t == NPAIR - 1))
        tail.append(("store", g))
    tail.append(("dma", None, 512, 1024))          # g1 + g5
    return slot_ops, tail


def _build_nc():
    nc = bacc.Bacc("TRN2", target_bir_lowering=False, debug=False,
                   num_devices=NCORES)

    x_d = nc.dram_tensor("x", [2, P, N], BF16, kind="ExternalInput")
    wqk_d = nc.dram_tensor("wqk", [P, 2 * P], BF16, kind="ExternalInput")
    wrest_d = nc.dram_tensor("wrest", [P, 2 * C], BF16, kind="ExternalInput")
    fblob_d = nc.dram_tensor("fblob", [P, C + 4], F32, kind="ExternalInput")
    ident_d = nc.dram_tensor("ident", [P, P], BF16, kind="ExternalInput")
    out_d = nc.dram_tensor("out", [2, P, N], BF16, kind="ExternalOutput")
    warm_d = nc.dram_tensor("warm", [P, 1], F32, kind="ExternalOutput")
    debug = bool(os.environ.get("KV2_DEBUG"))
    if debug:
        dbg_s_d = nc.dram_tensor("dbg_s", [P, NI, 4], F32,
                                 kind="ExternalOutput")
        dbg_vhi_d = nc.dram_tensor("dbg_vhi", [P, NI, C], F8,
                                   kind="ExternalOutput")
        dbg_vlo_d = nc.dram_tensor("dbg_vlo", [P, NI, C], F8,
                                   kind="ExternalOutput")
        dbg_ea_d = nc.dram_tensor("dbg_ea", [P, NI, EA], F8,
                                  kind="ExternalOutput")
        dbg_eb_d = nc.dram_tensor("dbg_eb", [P, NI, EB], F8,
                                  kind="ExternalOutput")
        dbg_vsb_d = nc.dram_tensor("dbg_vsb", [P, NI, C], BF16,
                                   kind="ExternalOutput")
        dbg_qk_d = nc.dram_tensor("dbg_qk", [CK, 2, N], BF16,
                                  kind="ExternalOutput")

    slot_ops, tail_ops = _plan_att()

    with tile.TileContext(nc) as tc:
        with (
            tc.tile_pool(name="consts", bufs=1) as consts,
            tc.tile_pool(name="big", bufs=1) as big,
            tc.tile_pool(name="work", bufs=6) as work,
            tc.tile_pool(name="ps_ea", bufs=1, space="PSUM") as ps_ea,
            tc.tile_pool(name="ps_eb", bufs=1, space="PSUM") as ps_eb,
            tc.tile_pool(name="ps_l", bufs=1, space="PSUM") as ps_l,
        ):
            # ---------------- PE warmup under the input DMAs --------------
            dummy = consts.tile([P, 512], BF16)
            nc.gpsimd.memset(dummy[:], 0)
            warm_sb = consts.tile([P, 1], F32)
            nc.scalar.activation(warm_sb[0:1, 0:1], dummy[0:1, 0:1], AF.Exp)
            psd = ps_l.tile([P, 512], F32, tag="L0", name="warm")
            for w in range(N_WARM):
                nc.tensor.matmul(psd[:], dummy[:, :P], dummy[:],
                                 start=(w == 0), stop=(w == N_WARM - 1))
            nc.vector.tensor_copy(warm_sb, psd[:, 0:1])

            # ---------------- inputs --------------------------------------
            # SP queue carries the critical x path (wqk + two x halves);
            # the small weight blobs ride the ACT DGE queue in parallel.
            xt = big.tile([P, 2, N], BF16)
            fblob = consts.tile([P, C + 4], F32)
            wqk = consts.tile([P, 2 * P], BF16)
            ident = consts.tile([P, P], BF16)
            wrest = consts.tile([P, 2 * C], BF16)
            x_r = x_d[:].rearrange("c p n -> p c n")
            nc.sync.dma_start(xt[:, :, 0:1024], x_r[:, :, 0:1024])
            nc.sync.dma_start(wqk[:], wqk_d[:])
            nc.sync.dma_start(xt[:, :, 1024:N], x_r[:, :, 1024:N])
            nc.scalar.dma_start(ident[:], ident_d[:])
            nc.scalar.dma_start(fblob, fblob_d[:])
            nc.scalar.dma_start(wrest[:], wrest_d[:])

            qb = fblob[0:CK, 0:1]
            kb = fblob[0:CK, 1:2]
            vb = fblob[:, 4:C + 4]
            gbias = fblob[:, 2:4]

            # ---------------- resident SBUF tensors -----------------------
            q_t = big.tile([CK, N], BF16)
            k_t = big.tile([CK, N], BF16)
            expA8 = big.tile([P, NI, EA], F8)
            expB8 = big.tile([P, NI, EB], F8)
            vsb = big.tile([P, NI, C], BF16)
            vhi8 = big.tile([P, NI, C], F8)
            vlo8 = big.tile([P, NI, C], F8)
            out_sb = big.tile([P, 2, N], BF16)
            out_r = out_d[:].rearrange("c p n -> p c n")
            s_all = big.tile([P, NI, 4], F32)      # sA, sB1+sB2, s, inv
            accs = {g: big.tile([P, 512], BF16, name=f"acc{g}")
                    for g in range(NG) if g not in FINAL_GROUPS}

            # ---------------- Q / K projections ---------------------------
            def psk_mms(ti, tag, on_act):
                j0, jw = QK_TILES[ti]
                psk = ps_l.tile([P, 512], F32, tag=tag, name=f"psk{ti}")
                for c in range(2):
                    nc.tensor.matmul(psk[:CK, :jw],
                                     wqk[:, c * P + CK:(c + 1) * P],
                                     xt[:, c, j0:j0 + jw],
                                     start=(c == 0), stop=(c == 1))
                if on_act:
                    nc.scalar.activation(k_t[:, j0:j0 + jw], psk[:CK, :jw],
                                         AF.Identity, bias=kb)
                else:
                    nc.vector.tensor_scalar_add(k_t[:, j0:j0 + jw],
                                                psk[:CK, :jw], kb)

            q_bias_pend = {}

            def psq_mms(ti, tag, on_act=False, defer_bias=False):
                j0, jw = QK_TILES[ti]
                psq = ps_l.tile([P, 512], F32, tag=tag, name=f"psq{ti}")
                for c in range(2):
                    nc.tensor.matmul(psq[:CK, :jw], wqk[:, c * P:c * P + CK],
                                     xt[:, c, j0:j0 + jw],
                                     start=(c == 0), stop=(c == 1))
                if on_act:
                    nc.scalar.activation(q_t[:, j0:j0 + jw], psq[:CK, :jw],
                                         AF.Identity, bias=qb)
                elif defer_bias:
                    q_bias_pend[ti] = psq
                else:
                    nc.vector.tensor_scalar_add(q_t[:, j0:j0 + jw],
                                                psq[:CK, :jw], qb)

            def flush_q_bias(ti):
                j0, jw = QK_TILES[ti]
                psq = q_bias_pend.pop(ti)
                nc.vector.tensor_scalar_add(q_t[:, j0:j0 + jw],
                                            psq[:CK, :jw], qb)

            # ---------------- energy --------------------------------------
            eps_of = {}

            def emit_energy(kk, part):
                width, subs, tag, pool = (
                    (EA, EA_SUBS, "EA", ps_ea) if part == 0
                    else (EB, EB_SUBS, "EB", ps_eb))
                base = 0 if part == 0 else EA
                eps = pool.tile([P, width], F32, tag=tag, name=f"eps{part}")
                for (o0, ow) in subs:
                    nc.tensor.matmul(
                        eps[:, o0:o0 + ow],
                        q_t[:, kk * P:(kk + 1) * P],
                        k_t[:, base + o0:base + o0 + ow],
                        start=True, stop=True)
                eps_of[part] = eps

            # ---------------- V projection batches ------------------------
            def emit_vbatch(bi):
                lane, _ = VB_SLOT[bi]
                psv = ps_l.tile([P, 512], F32, tag=f"L{lane}", name=f"vb{bi}")
                for half in range(2):
                    ci = 2 * bi + half
                    for c in range(2):
                        nc.tensor.matmul(
                            psv[:, half * C:(half + 1) * C],
                            xt[:, c, ci * P:(ci + 1) * P],
                            wrest[:, c * C:(c + 1) * C],
                            start=(half == 0 and c == 0),
                            stop=(half == 1 and c == 1))
                nc.vector.tensor_tensor(vsb[:, 2 * bi, :], psv[:, 0:C],
                                        vb, ALU.add)
                nc.vector.tensor_tensor(vsb[:, 2 * bi + 1, :], psv[:, C:2 * C],
                                        vb, ALU.add)

            # ---------------- per-chunk DVE/Pool chain --------------------
            def emit_sa_tree(k, t1):
                """A-half row-sum start: gpsimd pairwise add right after
                expA(k); the DVE reduce + combine happen in chunk_post."""
                nc.gpsimd.tensor_tensor(t1[:], expA8[:, k, 0:512],
                                        expA8[:, k, 512:1024], ALU.add)

            def emit_chunk_post(k, t1, vs, a_accum=False):
                """row sums + v hi/lo split for chunk k (after exps of k).
                s_all cols: 0 = sA (accum or reduce), 2 = s total, 3 = inv.
                sB always comes from expB's accum_out (col 1)."""
                if not a_accum:
                    nc.vector.tensor_reduce(s_all[:, k, 0:1], t1[:],
                                            axis=AX.X, op=ALU.add)
                nc.vector.tensor_tensor(s_all[:, k, 2:3], s_all[:, k, 0:1],
                                        s_all[:, k, 1:2], ALU.add)
                nc.vector.reciprocal(s_all[:, k, 3:4], s_all[:, k, 2:3])
                # vs = vsb * inv * SCALE ; v_hi = fp8(vs); v_lo = fp8(vs-v_hi)
                nc.vector.tensor_scalar(vs[:], vsb[:, k, :],
                                        s_all[:, k, 3:4], SCALE,
                                        ALU.mult, ALU.mult)
                nc.vector.tensor_copy(vhi8[:, k, :], vs[:])
                nc.vector.tensor_tensor(vlo8[:, k, :], vs[:], vhi8[:, k, :],
                                        ALU.subtract)

            # ---------------- att machinery -------------------------------
            def exp_slice(t, j0, jw):
                if j0 < EA:
                    return expA8[:, 2 * t:2 * t + 2, j0:j0 + jw]
                return expB8[:, 2 * t:2 * t + 2, j0 - EA:j0 - EA + jw]

            lane_tile = {}
            tail_shared = {}

            def _open_tile(tag):
                if tag == "EA2":
                    if "EA2" not in tail_shared:
                        tail_shared["EA2"] = ps_ea.tile([P, 1024], F32,
                                                        tag="EA", name="tEA")
                    return tail_shared["EA2"]
                if tag == "EB2":
                    if "EB2" not in tail_shared:
                        tail_shared["EB2"] = ps_eb.tile([P, 1024], F32,
                                                        tag="EB", name="tEB")
                    return tail_shared["EB2"]
                return ps_l.tile([P, 512], F32, tag=tag, name="lane")

            def do_att_op(op):
                kind = op[0]
                if kind == "open":
                    _, g, tag, off, reload = op
                    lt = _open_tile(tag)
                    lane_tile[g] = (lt, off)
                    if reload:
                        nc.tensor.matmul(lt[:, off:off + 512], ident[:],
                                         accs[g][:], start=True, stop=False)
                elif kind == "step":
                    _, g, t, st, sp = op
                    lt, off = lane_tile[g]
                    if g == G_COMP:
                        for oi in range(2):
                            for pi, v8 in enumerate((vhi8, vlo8)):
                                nc.tensor.matmul(
                                    lt[:, off + oi * 256:off + oi * 256 + 256],
                                    v8[:, 2 * t:2 * t + 2,
                                       oi * P:(oi + 1) * P],
                                    exp_slice(t, 2048, 256),
                                    start=(st and oi == 0 and pi == 0),
                                    stop=(sp and oi == 1 and pi == 1),
                                    perf_mode=DR)
                    else:
                        oc, j0 = G_STD[g]
                        for pi, v8 in enumerate((vhi8, vlo8)):
                            nc.tensor.matmul(
                                lt[:, off:off + 512],
                                v8[:, 2 * t:2 * t + 2, oc * P:(oc + 1) * P],
                                exp_slice(t, j0, 512),
                                start=(st and pi == 0),
                                stop=(sp and pi == 1),
                                perf_mode=DR)
                elif kind == "step1":
                    # single-chunk (non-DoubleRow) fp8 step: used to split
                    # the last pair so the chunk-16 half runs before the
                    # final exp/v-split completes.
                    _, g, ci, st, sp = op
                    lt, off = lane_tile[g]

                    def exp1(j0, jw):
                        if j0 < EA:
                            return expA8[:, ci, j0:j0 + jw]
                        return expB8[:, ci, j0 - EA:j0 - EA + jw]

                    if g == G_COMP:
                        for oi in range(2):
                            for pi, v8 in enumerate((vhi8, vlo8)):
                                nc.tensor.matmul(
                                    lt[:, off + oi * 256:off + oi * 256 + 256],
                                    v8[:, ci, oi * P:(oi + 1) * P],
                                    exp1(2048, 256),
                                    start=(st and oi == 0 and pi == 0),
                                    stop=(sp and oi == 1 and pi == 1))
                    else:
                        oc, j0 = G_STD[g]
                        for pi, v8 in enumerate((vhi8, vlo8)):
                            nc.tensor.matmul(
                                lt[:, off:off + 512],
                                v8[:, ci, oc * P:(oc + 1) * P],
                                exp1(j0, 512),
                                start=(st and pi == 0),
                                stop=(sp and pi == 1))
                elif kind == "flush":
                    _, g = op
                    lt, off = lane_tile[g]
                    nc.vector.tensor_copy(accs[g][:], lt[:, off:off + 512])
                elif kind == "store":
                    _, g = op
                    lt, off = lane_tile[g]
                    on_act = g in STORE_ON_ACT

                    def _st(dst_ap, src_ap, oc):
                        if on_act:
                            nc.scalar.activation(dst_ap, src_ap, AF.Identity,
                                                 bias=gbias[:, oc:oc + 1],
                                                 scale=1.0 / SCALE)
                        else:
                            nc.vector.tensor_scalar(
                                dst_ap, src_ap, 1.0 / SCALE,
                                gbias[:, oc:oc + 1], ALU.mult, ALU.add)

                    if g == G_COMP:
                        for oi, oc in enumerate((0, 1)):
                            _st(out_sb[:, oc, 2048:2304],
                                lt[:, off + oi * 256:off + oi * 256 + 256],
                                oc)
                    else:
                        oc, j0 = G_STD[g]
                        _st(out_sb[:, oc, j0:j0 + 512],
                            lt[:, off:off + 512], oc)
                elif kind == "dma":
                    _, oc, j0, j1 = op
                    if oc is None:
                        nc.sync.dma_start(out_r[:, :, j0:j1],
                                          out_sb[:, :, j0:j1])
                    else:
                        nc.sync.dma_start(out_d[oc, :, j0:j1],
                                          out_sb[:, oc, j0:j1])

            # ================== HEAD ======================================
            # K chain gates exp(0); k1 bias on ACT, rest on DVE.
            psk_mms(0, "L0", on_act=False)
            psq_mms(0, "L1", on_act=True)
            psk_mms(1, "L2", on_act=True)
            emit_energy(0, 0)
            psk_mms(2, "L0", on_act=False)
            psk_mms(3, "L1", on_act=False)
            psk_mms(4, "L2", on_act=False)
            emit_energy(0, 1)
            # q tiles 1-4: matmuls in head (lane rings), biases deferred
            psq_mms(1, "L0", defer_bias=True)
            psq_mms(2, "L1", defer_bias=True)
            flush_q_bias(1)
            emit_vbatch(0)

            # ================== PHASE =====================================
            vb_by_slot = {}
            for bi, (lane, s) in VB_SLOT.items():
                vb_by_slot.setdefault(s, []).append(bi)

            for k in range(NI):
                # ACT: exps of chunk k. sB via expB's accumulator (on the
                # pair-ready critical path); sA via gpsimd+DVE off ACT. The
                # last two chunks accum both so the final v-split is fast.
                a_accum = k >= NI - 2
                nc.scalar.activation(expA8[:, k, :], eps_of[0][:], AF.Exp,
                                     accum_out=(s_all[:, k, 0:1] if a_accum
                                                else None))
                t1 = work.tile([P, 512], BF16, tag=f"t1_{k % 2}")
                if not a_accum:
                    emit_sa_tree(k, t1)
                nc.scalar.activation(expB8[:, k, :], eps_of[1][:], AF.Exp,
                                     accum_out=s_all[:, k, 1:2])
                # DVE: rowsum combine + v split of chunk k
                vs = work.tile([P, C], BF16, tag=f"vs_{k % 2}")
                emit_chunk_post(k, t1, vs, a_accum=a_accum)
                if k == 1:
                    flush_q_bias(2)
                    nc.sync.dma_start(warm_d[:], warm_sb)
                if k == 3:
                    flush_q_bias(3)
                if k == 10:
                    flush_q_bias(4)
                # PE: energy(k+1)A first (it gates exp(k+1)A), a couple of
                # att steps, energy(k+1)B, then V batches and the rest.
                aops = slot_ops[k]
                # between energy A and B, only run steps whose pair data is
                # comfortably old — fresh-pair steps would stall the
                # in-order PE stream and delay energy B / the next exp.
                n_pre = 0
                for op in aops:
                    if op[0] == "open":
                        n_pre += 1
                    elif op[0] == "step" and op[2] <= (k - 4) // 2:
                        n_pre += 1
                    else:
                        break
                n_pre = min(n_pre, 5)
                if k + 1 < NI:
                    emit_energy(k + 1, 0)
                for op in aops[:n_pre]:
                    do_att_op(op)
                if k + 1 < NI:
                    emit_energy(k + 1, 1)
                for bi in vb_by_slot.get(k, []):
                    emit_vbatch(bi)
                if k == 2:
                    psq_mms(3, "L2", defer_bias=True)
                if k == 9:
                    psq_mms(4, "L2", defer_bias=True)
                for op in aops[n_pre:]:
                    do_att_op(op)

            # ================== TAIL ======================================
            for op in tail_ops:
                do_att_op(op)

            if debug:
                nc.sync.dma_start(dbg_s_d[:], s_all[:])
                nc.sync.dma_start(dbg_vhi_d[:], vhi8[:])
                nc.sync.dma_start(dbg_vlo_d[:], vlo8[:])
                nc.sync.dma_start(dbg_ea_d[:], expA8[:])
                nc.sync.dma_start(dbg_eb_d[:], expB8[:])
                nc.sync.dma_start(dbg_vsb_d[:], vsb[:])
                nc.sync.dma_start(dbg_qk_d[:, 0], q_t[:])
                nc.sync.dma_start(dbg_qk_d[:, 1], k_t[:])

    nc.compile()
    return nc


_NC_CACHE = []


def _get_nc():
    if not _NC_CACHE:
        _NC_CACHE.append(_build_nc())
    return _NC_CACHE[0]


def _prep_inputs(x, query_weight, query_bias, key_weight, key_bias,
                 value_weight, value_bias, gamma_weight, gamma_bias):
    bf16 = ml_dtypes.bfloat16
    x = np.asarray(x, np.float32).reshape(B, C, N)
    qw = np.asarray(query_weight, np.float32)[:, :, 0, 0]
    kw = np.asarray(key_weight, np.float32)[:, :, 0, 0]
    vw = np.asarray(value_weight, np.float32)[:, :, 0, 0]
    gw = np.asarray(gamma_weight, np.float32)[:, :, 0, 0]

    wcat_t = np.concatenate([qw, kw], axis=0).T
    wqk = np.ascontiguousarray(
        wcat_t.reshape(2, P, P).transpose(1, 0, 2).reshape(P, 2 * P))

    w_comb = (gw @ vw).T
    wrest = np.ascontiguousarray(
        w_comb.reshape(2, P, C).transpose(1, 0, 2).reshape(P, 2 * C))
    bvg = gw @ np.asarray(value_bias, np.float32)

    fblob = np.zeros((P, C + 4), np.float32)
    fblob[0:CK, 0] = np.asarray(query_bias, np.float32)
    fblob[0:CK, 1] = np.asarray(key_bias, np.float32)
    fblob[:, 2:4] = np.asarray(gamma_bias, np.float32).reshape(2, P).T
    fblob[:, 4:C + 4] = bvg[None, :]

    base = {
        "wqk": wqk.astype(bf16),
        "wrest": wrest.astype(bf16),
        "fblob": fblob,
        "ident": np.eye(P, dtype=np.float32).astype(bf16),
    }
    in_maps = []
    for b in range(B):
        m = dict(base)
        m["x"] = x[b].reshape(2, P, N).astype(bf16)
        in_maps.append(m)
    return in_maps


def kernel(x, query_weight, query_bias, key_weight, key_bias,
           value_weight, value_bias, gamma_weight, gamma_bias, k):
    assert int(k) == C // CK
    in_maps = _prep_inputs(x, query_weight, query_bias, key_weight, key_bias,
                           value_weight, value_bias, gamma_weight, gamma_bias)
    nc = _get_nc()
    res = run_bass_kernel_spmd(nc, in_maps, core_ids=list(range(NCORES)))

    out = np.empty((B, C, H, W), np.float32)
    for b in range(B):
        out[b] = np.asarray(res.results[b]["out"],
                            np.float32).reshape(C, H, W)
    return out
